# revision 13
# baseline (speedup 1.0000x reference)
"""Distributed AssociationLoss on 8 TRN2 NeuronCores.

Reference computation (N=8192 anchors, D=128, ids in [0,4000), cls=id//1000):
  d[i,j]   = euclidean cdist of associations
  hard_pos = max_j{same_id} d[j,i];  hard_neg = min_j{same_cls & !same_id} d[j,i]
  triplet  = relu(margin + hard_pos - hard_neg) gated on has_neg
  loss     = mean over present ids of (mean of triplet over id's anchors)

Kernel strategy (per core c of 8, shard = rows j in [c*1024,(c+1)*1024)):
  Work in squared distances; sqrt only after the global reductions
  (max/min commute with monotone sqrt(relu(.))).
  e[i,j] = -2 x_i.x_j + |x_j|^2  (the |x_i|^2 term is added after reduction)
  Band arithmetic folds the masks into ONE matrix per tile:
    mpos[i,j] = e - Q*same_cls + HUGE*same_id
  - max_j mpos  = HUGE - Q + max_{same_id} e        (top band)
  - min_j mpos  = -Q + min_{same_cls&!same_id} e    (bottom band, if any neg)
    and "has_neg" = (min_j mpos < -THRESH) globally.
  - same_cls enters via a K=5 fp32 matmul (cls one-hots scaled -Q, plus a
    ones row carrying |x_j|^2); same_id*HUGE either via DVE is_equal
    (mode 'dve') or via fp8 one-hot matmuls of (id%128, id//128) + ACT
    relu threshold (mode 'act').
  Counts per anchor (for the per-id mean) ride along as an accumulator:
    cnt_i = sum_j same_id[i,j]  -> segment mean weights w_i = 1/cnt_i,
    loss = sum_i trip_i*w_i / sum_i w_i  (identical to the reference's
    segment-sum formulation).
  Cross-core: ONE AllGather of [128,192] f32 per core (64 cols hard-pos
  partials | 64 cols negated mins | 64 cols counts), then a local 8-way
  reduce. Anchor i lives at (p,t) = (i%128, i//128) in all [128,64] maps.
"""

import os
import numpy as np

N = 8192
D = 128
M = 8
SH = N // M          # 1024 shard rows per core
T = N // 128         # 64 anchor tiles
HUGE = float(2 ** 17)
Q = float(2 ** 14)
THRESH = 8000.0
MARGIN = 0.1

MODE = os.environ.get("ASSOC_MODE", "act")       # 'act' or 'dve'
F32R = os.environ.get("ASSOC_F32R", "1") == "1"  # fast fp32 matmul mode

_CACHE = {}


def _build(mode, f32r):
    import concourse.bass as bass
    import concourse.bacc as bacc
    import concourse.mybir as mybir
    import concourse.tile as tile

    f32 = mybir.dt.float32
    fp8 = mybir.dt.float8e4
    alu = mybir.AluOpType
    act_f = mybir.ActivationFunctionType
    X = mybir.AxisListType.X

    nc = bacc.Bacc("TRN2", target_bir_lowering=False, debug=False, num_devices=M)

    f32r = mybir.dt.float32r
    xt2_d = nc.dram_tensor("xt2", [128, N], f32r, kind="ExternalInput")
    xts_d = nc.dram_tensor("xts", [128, SH], f32r, kind="ExternalInput")
    lsc_d = nc.dram_tensor("lsc", [5, N], f32r, kind="ExternalInput")
    rsc_d = nc.dram_tensor("rsc", [5, SH], f32r, kind="ExternalInput")
    if mode == "dve":
        idjb_d = nc.dram_tensor("idjb", [128, SH], f32, kind="ExternalInput")
        idpt_d = nc.dram_tensor("idpt", [128, T], f32, kind="ExternalInput")
    else:
        moh_d = nc.dram_tensor("moh", [128, N], fp8, kind="ExternalInput")
        doh_d = nc.dram_tensor("doh", [32, N], fp8, kind="ExternalInput")
        mohj_d = nc.dram_tensor("mohj", [128, SH], fp8, kind="ExternalInput")
        dohj_d = nc.dram_tensor("dohj", [32, SH], fp8, kind="ExternalInput")
    ones_d = nc.dram_tensor("ones", [128, 1], f32r, kind="ExternalInput")
    out_d = nc.dram_tensor("out", [1, 1], f32, kind="ExternalOutput")

    def r(ap):
        return ap

    with tile.TileContext(nc) as tc:
        with (
            tc.tile_pool(name="cst", bufs=1) as cst,
            tc.tile_pool(name="work", bufs=3) as work,
            tc.tile_pool(name="pp", bufs=2, space="PSUM") as pp,
            tc.tile_pool(name="pp2", bufs=2, space="PSUM") as pp2,
            tc.tile_pool(name="dram", bufs=1, space="DRAM") as dpool,
        ):
            # ---- persistent inputs ----
            xt2c = []
            for k in range(8):
                c = cst.tile([128, SH], f32r, tag=f"xt2_{k}")
                nc.sync.dma_start(c[:], xt2_d.ap()[:, k * SH:(k + 1) * SH])
                xt2c.append(c)
            xts = cst.tile([128, SH], f32r, tag="xts")
            nc.sync.dma_start(xts[:], xts_d.ap())
            lsc = cst.tile([5, N], f32r, tag="lsc")
            nc.sync.dma_start(lsc[:], lsc_d.ap())
            rsc = cst.tile([5, SH], f32r, tag="rsc")
            nc.sync.dma_start(rsc[:], rsc_d.ap())
            if mode == "dve":
                idjb = cst.tile([128, SH], f32, tag="idjb")
                nc.sync.dma_start(idjb[:], idjb_d.ap())
                idpt = cst.tile([128, T], f32, tag="idpt")
                nc.sync.dma_start(idpt[:], idpt_d.ap())
            else:
                moh = cst.tile([128, N], fp8, tag="moh")
                nc.sync.dma_start(moh[:], moh_d.ap())
                doh = cst.tile([32, N], fp8, tag="doh")
                nc.sync.dma_start(doh[:], doh_d.ap())
                mohj = cst.tile([128, SH], fp8, tag="mohj")
                nc.sync.dma_start(mohj[:], mohj_d.ap())
                dohj = cst.tile([32, SH], fp8, tag="dohj")
                nc.sync.dma_start(dohj[:], dohj_d.ap())

            ones128 = cst.tile([128, 1], f32r, tag="ones")
            nc.sync.dma_start(ones128[:], ones_d.ap())

            # ---- |x_j|^2 of the shard -> rsc row 4 (feeds the K=5 matmul) ----
            xsq = work.tile([128, SH], f32r, tag="mpos")
            nc.scalar.square(xsq[:], xts[:].bitcast(f32))
            for c in range(2):
                sl = slice(c * 512, (c + 1) * 512)
                pq = pp.tile([1, 512], f32, tag="ps1")
                nc.tensor.matmul(pq[:], r(ones128[:]), r(xsq[:, sl]),
                                 start=True, stop=True)
                sqst = work.tile([1, 512], f32r, tag="sqst", name=f"sqst{c}")
                nc.scalar.copy(sqst[:], pq[:])
                nc.sync.dma_start(rsc[4:5, sl], sqst[:])

            # ---- |x_i|^2 for all anchors, laid out [128 (i%128), 64 (i//128)] ----
            sqrow = cst.tile([1, N], f32, tag="sqrow")
            for k in range(8):
                xsqf = work.tile([128, SH], f32r, tag="mpos")
                nc.scalar.square(xsqf[:], xt2c[k][:].bitcast(f32))
                for c in range(2):
                    sl = slice(c * 512, (c + 1) * 512)
                    pq = pp.tile([1, 512], f32, tag="ps1")
                    nc.tensor.matmul(pq[:], r(ones128[:]), r(xsqf[:, sl]),
                                     start=True, stop=True)
                    # xt2 = -2x so the ones-matmul gives 4|x|^2; scale by 1/4
                    nc.scalar.mul(sqrow[0:1, k * SH + c * 512:k * SH + (c + 1) * 512],
                                  pq[:], 0.25)
            sqd = dpool.tile([N], f32)
            nc.sync.dma_start(sqd[:], sqrow[:])
            sq_pt = cst.tile([128, T], f32, tag="sqpt")
            nc.sync.dma_start(sq_pt[:], sqd[:].rearrange("(t p) -> p t", p=128))

            # ---- per-shard stats ----
            HP = cst.tile([128, T], f32, tag="HP")
            HN = cst.tile([128, T], f32, tag="HN")
            CNTH = cst.tile([128, T], f32, tag="CNTH")

            for t in range(T):
                k8, r8 = divmod(t, 8)
                lhs_xy = xt2c[k8][:, r8 * 128:(r8 + 1) * 128]
                ps1 = pp.tile([128, SH], f32, tag="ps1")
                for c in range(2):
                    sl = slice(c * 512, (c + 1) * 512)
                    nc.tensor.matmul(ps1[:, sl], r(lhs_xy), r(xts[:, sl]),
                                     start=True, stop=False)
                    nc.tensor.matmul(ps1[:, sl], r(lsc[:, t * 128:(t + 1) * 128]),
                                     r(rsc[:, sl]), start=False, stop=True)
                midh = work.tile([128, SH], f32, tag="midh")
                if mode == "dve":
                    m01 = work.tile([128, SH], f32, tag="m01")
                    nc.vector.tensor_scalar(
                        m01[:], idjb[:], idpt[:, t:t + 1], None,
                        alu.is_equal, alu.add, accum_out=CNTH[:, t:t + 1])
                    nc.vector.tensor_scalar(midh[:], m01[:], HUGE, None,
                                            alu.mult)
                else:
                    ps2 = pp2.tile([128, SH], f32, tag="ps2")
                    for c in range(2):
                        sl = slice(c * 512, (c + 1) * 512)
                        nc.tensor.matmul(ps2[:, sl], moh[:, t * 128:(t + 1) * 128],
                                         mohj[:, sl], start=True, stop=False)
                        nc.tensor.matmul(ps2[:, sl], doh[:, t * 128:(t + 1) * 128],
                                         dohj[:, sl], start=False, stop=True)
                    # relu(HUGE*(A+B) - HUGE): {0,1}->0, 2->HUGE; accum=HUGE*cnt
                    nc.scalar.activation(midh[:], ps2[:], act_f.Relu,
                                         bias=-HUGE, scale=HUGE,
                                         accum_out=CNTH[:, t:t + 1])
                mpos = work.tile([128, SH], f32, tag="mpos")
                nc.vector.tensor_tensor(mpos[:], ps1[:], midh[:], alu.add)
                nc.vector.tensor_reduce(HP[:, t:t + 1], mpos[:], axis=X,
                                        op=alu.max)
                nc.vector.tensor_reduce(HN[:, t:t + 1], mpos[:], axis=X,
                                        op=alu.min)

            # ---- cross-core exchange: one AllGather of [128,192] ----
            MHN = cst.tile([128, T], f32, tag="MHN")
            nc.vector.tensor_scalar(MHN[:], HN[:], -1.0, None, alu.mult)
            cc_in = dpool.tile([128, 3 * T], f32)
            cc_out = dpool.tile([M * 128, 3 * T], f32)
            nc.sync.dma_start(cc_in[:][:, 0:T], HP[:])
            nc.sync.dma_start(cc_in[:][:, T:2 * T], MHN[:])
            nc.sync.dma_start(cc_in[:][:, 2 * T:3 * T], CNTH[:])
            if os.environ.get("ASSOC_NOCC", "0") == "1":
                for rr in range(M):
                    nc.sync.dma_start(cc_out[:][rr * 128:(rr + 1) * 128, :],
                                      cc_in[:])
            else:
                nc.gpsimd.collective_compute(
                    "AllGather", alu.bypass,
                    replica_groups=[list(range(M))],
                    ins=[cc_in[:].opt()],
                    outs=[cc_out[:].opt()],
                )
            # one contiguous DMA: rank-r rows p, 128p+r -> partition p, free (r, c)
            gat = cc_out[:].rearrange("(r p) c -> p r c", r=M)
            all_sb = work.tile([128, M * 3 * T], f32, tag="gath")
            nc.sync.dma_start(all_sb[:], gat)
            # strided views [128, c, r] so the 8-way rank reduce is innermost
            allv = all_sb[:].rearrange("p (r c) -> p c r", r=M)

            HPg = cst.tile([128, T], f32, tag="HPg")
            nc.vector.tensor_reduce(HPg[:], allv[:, 0:T, :], axis=X, op=alu.max)
            MHNg = cst.tile([128, T], f32, tag="MHNg")
            nc.vector.tensor_reduce(MHNg[:], allv[:, T:2 * T, :], axis=X,
                                    op=alu.max)
            CNTg = cst.tile([128, T], f32, tag="CNTg")
            nc.vector.tensor_reduce(CNTg[:], allv[:, 2 * T:3 * T, :], axis=X,
                                    op=alu.add)

            # ---- epilogue (all [128,64] elementwise) ----
            _ctr = [0]

            def wt():
                _ctr[0] += 1
                return work.tile([128, T], f32, tag=f"ep{_ctr[0]}", bufs=1,
                                 name=f"ep{_ctr[0]}")

            t1 = wt(); nc.vector.tensor_tensor(t1[:], HPg[:], sq_pt[:], alu.add)
            t2 = wt(); nc.vector.tensor_scalar(t2[:], t1[:], Q - HUGE, 0.0,
                                               alu.add, alu.max)
            hp = wt(); nc.scalar.sqrt(hp[:], t2[:])
            u1 = wt(); nc.vector.tensor_tensor(u1[:], sq_pt[:], MHNg[:],
                                               alu.subtract)
            u2 = wt(); nc.vector.tensor_scalar(u2[:], u1[:], Q, 0.0,
                                               alu.add, alu.max)
            hn = wt(); nc.scalar.sqrt(hn[:], u2[:])
            gate = wt(); nc.vector.tensor_scalar(gate[:], MHNg[:], THRESH, None,
                                                 alu.is_gt)
            dd = wt(); nc.vector.tensor_tensor(dd[:], hp[:], hn[:], alu.subtract)
            d2 = wt(); nc.vector.tensor_scalar(d2[:], dd[:], MARGIN, 0.0,
                                               alu.add, alu.max)
            trip = wt(); nc.vector.tensor_tensor(trip[:], d2[:], gate[:], alu.mult)
            # w = 1/cnt = HUGE/CNTg with one Newton step on the reciprocal
            r0 = wt(); nc.vector.reciprocal(r0[:], CNTg[:])
            aa = wt(); nc.vector.tensor_tensor(aa[:], CNTg[:], r0[:], alu.mult)
            bb = wt(); nc.vector.tensor_scalar(bb[:], aa[:], -1.0, 2.0,
                                               alu.mult, alu.add)
            r1 = wt(); nc.vector.tensor_tensor(r1[:], r0[:], bb[:], alu.mult)
            cnt_scale = 1.0 if mode == "dve" else HUGE
            w = wt(); nc.vector.tensor_scalar(w[:], r1[:], cnt_scale, None,
                                              alu.mult)

            SS = cst.tile([128, 2], f32, tag="SS")
            tw = wt()
            nc.vector.tensor_tensor(tw[:], trip[:], w[:], alu.mult)
            nc.vector.tensor_reduce(SS[:, 0:1], tw[:], axis=X, op=alu.add)
            nc.vector.tensor_reduce(SS[:, 1:2], w[:], axis=X, op=alu.add)
            psF = pp.tile([1, 2], f32, tag="ps1")
            nc.tensor.matmul(psF[:], ones128[:].bitcast(f32), SS[:], start=True, stop=True)

            fin = cst.tile([1, 8], f32, tag="fin")
            nc.vector.reciprocal(fin[0:1, 0:1], psF[0:1, 1:2])
            nc.vector.tensor_tensor(fin[0:1, 1:2], psF[0:1, 1:2],
                                    fin[0:1, 0:1], alu.mult)
            nc.vector.tensor_scalar(fin[0:1, 2:3], fin[0:1, 1:2], -1.0, 2.0,
                                    alu.mult, alu.add)
            nc.vector.tensor_tensor(fin[0:1, 3:4], fin[0:1, 0:1],
                                    fin[0:1, 2:3], alu.mult)
            nc.vector.tensor_tensor(fin[0:1, 4:5], psF[0:1, 0:1],
                                    fin[0:1, 3:4], alu.mult)
            nc.sync.dma_start(out_d.ap(), fin[0:1, 4:5])

    nc.compile()
    return nc


def _prep_inputs(associations, detection_ids, mode):
    import ml_dtypes

    x = np.ascontiguousarray(associations, dtype=np.float32)
    ids = np.asarray(detection_ids).astype(np.int64)
    idsf = ids.astype(np.float32)
    cls = (ids // 1000).astype(np.int64)

    xt2 = np.ascontiguousarray((-2.0 * x).T)                       # [128, N]
    lsc = np.zeros((5, N), np.float32)
    lsc[cls, np.arange(N)] = -Q
    lsc[4, :] = 1.0

    common = {"xt2": xt2, "lsc": lsc,
              "ones": np.ones((128, 1), np.float32)}
    if mode == "dve":
        common["idpt"] = np.ascontiguousarray(idsf.reshape(T, 128).T)
    else:
        a = (ids % 128).astype(np.int64)
        b = (ids // 128).astype(np.int64)
        moh = np.zeros((128, N), ml_dtypes.float8_e4m3fn)
        moh[a, np.arange(N)] = 1.0
        doh = np.zeros((32, N), ml_dtypes.float8_e4m3fn)
        doh[b, np.arange(N)] = 1.0
        common["moh"] = moh
        common["doh"] = doh

    in_maps = []
    for c in range(M):
        sl = slice(c * SH, (c + 1) * SH)
        m = dict(common)
        m["xts"] = np.ascontiguousarray(x[sl].T)                   # [128, SH]
        rsc = np.zeros((5, SH), np.float32)
        rsc[cls[sl], np.arange(SH)] = 1.0
        m["rsc"] = rsc
        if mode == "dve":
            m["idjb"] = np.ascontiguousarray(
                np.broadcast_to(idsf[sl], (128, SH)))
        else:
            a = (ids[sl] % 128).astype(np.int64)
            b = (ids[sl] // 128).astype(np.int64)
            mohj = np.zeros((128, SH), ml_dtypes.float8_e4m3fn)
            mohj[a, np.arange(SH)] = 1.0
            dohj = np.zeros((32, SH), ml_dtypes.float8_e4m3fn)
            dohj[b, np.arange(SH)] = 1.0
            m["mohj"] = mohj
            m["dohj"] = dohj
        in_maps.append(m)
    return in_maps


def _get_nc(mode, f32r):
    key = (mode, f32r)
    if key not in _CACHE:
        _CACHE[key] = _build(mode, f32r)
    return _CACHE[key]


def _ensure_ntff_hook():
    """The agent image lacks ``antenv.axon_hooks``; synthesize it and
    register the ctypes NTFF hook so trace=True works."""
    import sys
    import types

    try:
        from antenv.axon_hooks import get_axon_ntff_profile_hook  # noqa: F401
        return
    except ImportError:
        pass
    import antenv
    from trn_agent_boot.trn_boot import _ntff_profile_via_ctypes

    mod = types.ModuleType("antenv.axon_hooks")
    holder = {"h": None}
    mod.set_axon_ntff_profile_hook = lambda h: holder.__setitem__("h", h)
    mod.get_axon_ntff_profile_hook = lambda: holder["h"]
    sys.modules["antenv.axon_hooks"] = mod
    antenv.axon_hooks = mod
    mod.set_axon_ntff_profile_hook(
        _ntff_profile_via_ctypes("/opt/axon/libaxon_pjrt.so"))

    import concourse.bass_utils as bu
    bu.upload_artifacts = lambda tmpdir: f"local://{tmpdir}"


def kernel(associations, detection_ids, _trace=False):
    from concourse.bass_utils import run_bass_kernel_spmd

    if _trace:
        _ensure_ntff_hook()

    nc = _get_nc(MODE, F32R)
    in_maps = _prep_inputs(associations, detection_ids, MODE)
    res = run_bass_kernel_spmd(nc, in_maps, core_ids=list(range(M)),
                               trace=_trace)
    kernel.last_results = res
    val = res.results[0]["out"].reshape(())
    return np.asarray(val, dtype=np.float32)
